# revision 1
# baseline (speedup 1.0000x reference)
"""Trainium2 Bass kernel for the dual-branch agent-attention module.

Sharding: data-parallel over B=8 (one batch element per NeuronCore).
All transposes and weight permutations are done host-side; on-device
work is a streamed bf16 pipeline:

  prep:      agent projections k_ag/qa -> block-diagonal tiles; then
             effective score weights Weff_A = Wq @ k12bd and
             Weff_B = Wkhf @ qabd (associativity: the big activations
             never materialize q or kh at all).
  phase B:   v = attnT^T@Wv (with ones col), tT scores directly from
             attnT via Weff_B -> exp -> xs accumulation (softmax denom
             folded in via the ones column of v_aug).
  phase AC:  sT scores directly from xT via Weff_A -> exp -> PA,
             x_out with ones-column denom, normalize, PE-transpose,
             proj.

Head-major layout trick: the 2C projection outputs are permuted host-
side from (branch, head, d) to (head, branch, d) so each head pair
occupies one 128-partition tile; branch score scales (wa/wb * D^-0.5)
are folded into the K-side weights, so both branches' score maps come
out of a single contraction per head pair.

Bias handling: k-side biases that are constant along a softmax axis
cancel exactly and are dropped (bk_hf entirely; q-side bias of branch
A survives as the per-agent term c_A = k12bd^T @ bq, applied as the
exp's per-partition bias together with ba).
"""

import os
import sys
import numpy as np

for _p in ("/opt/trn_rl_repo", os.path.expanduser("~/.axon_site/_ro/trn_rl_repo")):
    if os.path.isdir(_p) and _p not in sys.path:
        sys.path.insert(0, _p)

import ml_dtypes

import concourse.bass as bass
import concourse.bacc as bacc
import concourse.tile as tile
from concourse import mybir
from concourse.bass_utils import run_bass_kernel_spmd
from concourse.masks import make_identity

BF16 = mybir.dt.bfloat16
F32 = mybir.dt.float32
NPBF16 = ml_dtypes.bfloat16

B, N, NA, H, D = 8, 4096, 64, 12, 32
C = H * D            # 384
C2 = 2 * C           # 768
NP = H // 2          # 6 head pairs
CH = 512             # seq chunk
NCH = N // CH        # 8
TPC = CH // 128      # 4 seq tiles per chunk
SCALE = D ** -0.5

_CACHE = {}


def _build_bass(finalize=True, zero_bias=False):
    nc = bacc.Bacc()

    # ---- DRAM I/O ----
    xT = nc.dram_tensor("xT", [C, N], BF16, kind="ExternalInput")
    attnT = nc.dram_tensor("attnT", [C, N], BF16, kind="ExternalInput")
    agT = nc.dram_tensor("agT", [C, NA], BF16, kind="ExternalInput")
    wqT = nc.dram_tensor("wqT", [C2, C], BF16, kind="ExternalInput")
    wkag = nc.dram_tensor("wkag", [C, C2], BF16, kind="ExternalInput")
    wqag = nc.dram_tensor("wqag", [C, C2], BF16, kind="ExternalInput")
    wkhfT = nc.dram_tensor("wkhfT", [C2, C], BF16, kind="ExternalInput")
    wv = nc.dram_tensor("wv", [C, H * 33], BF16, kind="ExternalInput")
    wproj = nc.dram_tensor("wproj", [C, C], BF16, kind="ExternalInput")
    bq = nc.dram_tensor("bq", [C2], F32, kind="ExternalInput")
    bkag = nc.dram_tensor("bkag", [C2], F32, kind="ExternalInput")
    bqag = nc.dram_tensor("bqag", [C2], F32, kind="ExternalInput")
    bv = nc.dram_tensor("bv", [H * 33], F32, kind="ExternalInput")
    bproj = nc.dram_tensor("bproj", [C], F32, kind="ExternalInput")
    bab = nc.dram_tensor("bab", [2], F32, kind="ExternalInput")
    out = nc.dram_tensor("out", [N, C], F32, kind="ExternalOutput")

    Exp = mybir.ActivationFunctionType.Exp

    def bcast_dram(ap, parts, cols):
        return bass.AP(tensor=ap.tensor, offset=ap.offset, ap=[[0, parts], [1, cols]])

    with tile.TileContext(nc) as tc:
        with (
            tc.tile_pool(name="const", bufs=1) as const,
            tc.tile_pool(name="inp", bufs=4) as p_in,
            tc.tile_pool(name="vv", bufs=2) as p_v,
            tc.tile_pool(name="pt", bufs=4) as p_pt,
            tc.tile_pool(name="pa", bufs=2) as p_pa,
            tc.tile_pool(name="xon", bufs=2) as p_xon,
            tc.tile_pool(name="xot", bufs=4) as p_xot,
            tc.tile_pool(name="osb", bufs=3) as p_out,
            tc.tile_pool(name="sm", bufs=4) as p_sm,
            tc.tile_pool(name="psA", bufs=2, space="PSUM") as psA,
            tc.tile_pool(name="psB", bufs=2, space="PSUM") as psB,
            tc.tile_pool(name="psC", bufs=2, space="PSUM") as psC,
            tc.tile_pool(name="psX", bufs=2, space="PSUM") as psX,
        ):
            # ---- constants ----
            w_qT = const.tile([128, 6, C], BF16)
            w_khfT = const.tile([128, 6, C], BF16)
            w_kag = const.tile([128, 3, C2], BF16)
            w_qag = const.tile([128, 3, C2], BF16)
            w_v = const.tile([128, 3, H * 33], BF16)
            w_pr = const.tile([128, 3, C], BF16)
            for dst, src in ((w_kag, wkag), (w_qag, wqag), (w_v, wv),
                             (w_qT, wqT), (w_khfT, wkhfT)):
                nc.sync.dma_start(out=dst, in_=src.rearrange("(k p) m -> p k m", p=128))
            b_q = const.tile([128, 6], F32)
            b_kag = const.tile([128, 6], F32)
            b_qag = const.tile([128, 6], F32)
            for dst, src in ((b_q, bq), (b_kag, bkag), (b_qag, bqag)):
                nc.gpsimd.dma_start(out=dst, in_=src.rearrange("(j p) -> p j", p=128))
            bv_row = const.tile([1, H * 33], BF16)
            nc.gpsimd.dma_start(out=bv_row, in_=bv[:].unsqueeze(0))
            bpr_row = const.tile([1, C], BF16)
            nc.gpsimd.dma_start(out=bpr_row, in_=bproj[:].unsqueeze(0))
            ones_row = const.tile([1, 128], BF16)
            nc.vector.memset(ones_row, 1.0)
            ones12 = const.tile([1, 12], BF16)
            nc.vector.memset(ones12, 1.0)
            ba_t = const.tile([128, 1], F32)
            nc.gpsimd.dma_start(out=ba_t, in_=bass.AP(tensor=bab[:].tensor, offset=0,
                                                      ap=[[0, 128], [1, 1]]))
            bb_t = const.tile([128, 1], F32)
            nc.gpsimd.dma_start(out=bb_t, in_=bass.AP(tensor=bab[:].tensor, offset=1,
                                                      ap=[[0, 128], [1, 1]]))
            ident = const.tile([128, 128], BF16)
            make_identity(nc, ident)
            ag_t = const.tile([128, 3, NA], BF16)
            nc.gpsimd.dma_start(out=ag_t, in_=agT.rearrange("(k p) m -> p k m", p=128))

            # Pre-touch DMA-loaded constants with tiny reads so wide ops
            # downstream only carry the PE wait.
            touch = const.tile([128, 16], F32)
            for i, t_ap in enumerate((b_q[:, 0:1], b_kag[:, 0:1], b_qag[:, 0:1],
                                      ba_t[:, 0:1], bb_t[:, 0:1])):
                nc.vector.tensor_copy(touch[:, i:i + 1], t_ap)
            nc.scalar.copy(touch[:, 8:9], ba_t[:, 0:1])
            nc.scalar.copy(touch[:, 9:10], bb_t[:, 0:1])

            # ---- prep: k_ag / qa projections -> block-diag tiles ----
            kag_sb = const.tile([128, 6, NA], BF16)
            qa_sb = const.tile([128, 6, NA], BF16)
            for w_t, b_t, dst in ((w_kag, b_kag, kag_sb), (w_qag, b_qag, qa_sb)):
                for j in range(6):
                    ps = psC.tile([128, NA], F32, tag="small")
                    for k in range(3):
                        nc.tensor.matmul(ps, lhsT=w_t[:, k, j * 128:(j + 1) * 128],
                                         rhs=ag_t[:, k, :], start=(k == 0), stop=(k == 2))
                    nc.vector.tensor_add(dst[:, j, :], ps,
                                         b_t[:, j:j + 1].to_broadcast([128, NA]))
            k12bd = const.tile([128, 6, 128], BF16)
            qabd = const.tile([128, 6, 128], BF16)
            for src, dst in ((kag_sb, k12bd), (qa_sb, qabd)):
                nc.vector.memset(dst, 0.0)
                for j in range(6):
                    nc.vector.tensor_copy(dst[0:64, j, 0:64], src[0:64, j, :])
                    nc.vector.tensor_copy(dst[64:128, j, 64:128], src[64:128, j, :])

            # ---- prep: effective score weights + branch-A exp bias ----
            weff_a = const.tile([128, 3, C2], BF16)
            weff_b = const.tile([128, 3, C2], BF16)
            for j in range(6):
                for k in range(3):
                    ps = psC.tile([128, 128], F32, tag="small")
                    nc.tensor.matmul(ps, lhsT=w_qT[:, j, k * 128:(k + 1) * 128],
                                     rhs=k12bd[:, j, :], start=True, stop=True)
                    nc.vector.tensor_copy(weff_a[:, k, j * 128:(j + 1) * 128], ps)
                    ps2 = psC.tile([128, 128], F32, tag="small")
                    nc.tensor.matmul(ps2, lhsT=w_khfT[:, j, k * 128:(k + 1) * 128],
                                     rhs=qabd[:, j, :], start=True, stop=True)
                    nc.scalar.copy(weff_b[:, k, j * 128:(j + 1) * 128], ps2)
            cba = None
            if not zero_bias:
                b_q_bf = const.tile([128, 6], BF16)
                nc.vector.tensor_copy(b_q_bf, b_q)
                cba = const.tile([128, 6], F32)
                for j in range(6):
                    ps = psC.tile([128, 1], F32, tag="small")
                    nc.tensor.matmul(ps, lhsT=k12bd[:, j, :], rhs=b_q_bf[:, j:j + 1],
                                     start=True, stop=True)
                    nc.vector.tensor_add(cba[:, j:j + 1], ps, ba_t[:, 0:1])

            # ---- phase B: values + branch-B attention ----
            xs_sb = const.tile([128, 6 * 66], F32)
            nc.vector.memset(xs_sb, 0.0)
            for c in range(NCH):
                at_t = p_in.tile([128, 3, CH], BF16, tag="inp")
                nc.sync.dma_start(
                    out=at_t,
                    in_=attnT.rearrange("(k p) s -> p k s", p=128)[:, :, c * CH:(c + 1) * CH])
                v_t = p_v.tile([128, TPC, H * 33], BF16)
                for t in range(TPC):
                    ps = psB.tile([128, H * 33], F32, tag="mid")
                    for k in range(3):
                        nc.tensor.matmul(ps, lhsT=at_t[:, k, t * 128:(t + 1) * 128],
                                         rhs=w_v[:, k, :], start=(k == 0), stop=False)
                    if zero_bias:
                        ps33 = ps.rearrange("p (h c) -> p h c", c=33)
                        nc.tensor.matmul(ps33[:, :, 32], lhsT=ones_row[:, :],
                                         rhs=ones12[:, :], start=False, stop=True)
                    else:
                        nc.tensor.matmul(ps, lhsT=ones_row[:, :], rhs=bv_row[:, :],
                                         start=False, stop=True)
                    nc.scalar.copy(v_t[:, t, :], ps)
                for t in range(TPC):
                    xs_stp = psX.tile([128, 6 * 66], F32, tag="xs")
                    ps4 = psC.tile([128, 512], F32, tag="small")
                    ps2 = psC.tile([128, 256], F32, tag="small")
                    for j in range(6):
                        dst = ps4[:, j * 128:(j + 1) * 128] if j < 4 else \
                            ps2[:, (j - 4) * 128:(j - 3) * 128]
                        for k in range(3):
                            nc.tensor.matmul(dst, lhsT=at_t[:, k, t * 128:(t + 1) * 128],
                                             rhs=weff_b[:, k, j * 128:(j + 1) * 128],
                                             start=(k == 0), stop=(k == 2))
                    pt4 = p_pt.tile([128, 512], BF16, tag="pt4")
                    pt2 = p_pt.tile([128, 256], BF16, tag="pt2")
                    bbias = 0.0 if zero_bias else bb_t[:, 0:1]
                    nc.scalar.activation(pt4, ps4, Exp, bias=bbias)
                    nc.scalar.activation(pt2, ps2, Exp, bias=bbias)
                    for j in range(6):
                        lhsT = pt4[:, j * 128:(j + 1) * 128] if j < 4 else \
                            pt2[:, (j - 4) * 128:(j - 3) * 128]
                        nc.tensor.matmul(xs_stp[:, j * 66:(j + 1) * 66], lhsT=lhsT,
                                         rhs=v_t[:, t, j * 66:(j + 1) * 66],
                                         start=True, stop=True)
                    nc.vector.tensor_add(xs_sb, xs_stp, xs_sb)

            nc.sync.dma_start(out=w_pr, in_=wproj.rearrange("(k p) m -> p k m", p=128))

            # ---- xs normalize -> block-diag [xs | 1] tiles ----
            xs_bd = const.tile([128, 6 * 66], BF16)
            xs3 = xs_sb[:].rearrange("p (j c) -> p j c", c=66)
            bd3 = xs_bd[:].rearrange("p (j c) -> p j c", c=66)
            nc.vector.memset(xs_bd, 0.0)
            nc.vector.memset(bd3[0:64, :, 32:33], 1.0)
            nc.vector.memset(bd3[64:128, :, 65:66], 1.0)
            rec6 = p_sm.tile([128, 6], F32, tag="rec")
            nc.vector.reciprocal(rec6[0:64, :], xs3[0:64, :, 32])
            nc.vector.reciprocal(rec6[64:128, :], xs3[64:128, :, 65])
            nc.vector.tensor_mul(bd3[0:64, :, 0:32], xs3[0:64, :, 0:32],
                                 rec6[0:64, :].unsqueeze(2).to_broadcast([64, 6, 32]))
            nc.vector.tensor_mul(bd3[64:128, :, 33:65], xs3[64:128, :, 33:65],
                                 rec6[64:128, :].unsqueeze(2).to_broadcast([64, 6, 32]))

            # ---- phase AC: branch-A attention + proj ----
            for c in range(NCH):
                xt_t = p_in.tile([128, 3, CH], BF16, tag="inp")
                nc.sync.dma_start(
                    out=xt_t,
                    in_=xT.rearrange("(k p) s -> p k s", p=128)[:, :, c * CH:(c + 1) * CH])
                pa_t = p_pa.tile([128, 6, CH], BF16)
                for j in range(6):
                    ps = psA.tile([128, CH], F32, tag="big")
                    for k in range(3):
                        nc.tensor.matmul(ps, lhsT=weff_a[:, k, j * 128:(j + 1) * 128],
                                         rhs=xt_t[:, k, :], start=(k == 0), stop=(k == 2))
                    nc.scalar.activation(pa_t[:, j, :], ps, Exp,
                                         bias=(0.0 if zero_bias else cba[:, j:j + 1]))
                for t in range(TPC):
                    xo_ps = psB.tile([128, 12 * 33], F32, tag="mid")
                    for j in range(6):
                        nc.tensor.matmul(xo_ps[:, j * 66:(j + 1) * 66],
                                         lhsT=pa_t[:, j, t * 128:(t + 1) * 128],
                                         rhs=xs_bd[:, j * 66:(j + 1) * 66],
                                         start=True, stop=True)
                    xo3 = xo_ps.rearrange("p (k c) -> p k c", c=33)
                    rec = p_sm.tile([128, 12], F32, tag="rec12")
                    nc.vector.reciprocal(rec, xo3[:, :, 32])
                    xon = p_xon.tile([128, C], BF16)
                    nc.vector.tensor_mul(xon[:].rearrange("p (k c) -> p k c", c=32),
                                         xo3[:, :, 0:32],
                                         rec[:].unsqueeze(2).to_broadcast([128, 12, 32]))
                    pr_ps = psX.tile([128, C], F32, tag="xs")
                    for f in range(3):
                        tp = psC.tile([128, 128], BF16, tag="small")
                        nc.tensor.transpose(tp, xon[:, f * 128:(f + 1) * 128], ident)
                        xot = p_xot.tile([128, 128], BF16)
                        nc.vector.tensor_copy(xot, tp)
                        nc.tensor.matmul(pr_ps, lhsT=xot, rhs=w_pr[:, f, :],
                                         start=(f == 0),
                                         stop=(zero_bias and f == 2),
                                         skip_group_check=True)
                    if not zero_bias:
                        nc.tensor.matmul(pr_ps, lhsT=ones_row[:, :], rhs=bpr_row[:, :],
                                         start=False, stop=True, skip_group_check=True)
                    o_sb = p_out.tile([128, C], F32)
                    nc.scalar.copy(o_sb, pr_ps)
                    r0 = (c * TPC + t) * 128
                    nc.sync.dma_start(out=out[r0:r0 + 128, :], in_=o_sb)
    if finalize:
        nc.finalize()
    return nc


def _prep_host(inputs):
    f32 = np.float32
    x = np.asarray(inputs["x"], f32)
    attn = np.asarray(inputs["attn"], f32)
    agent = np.asarray(inputs["agent_input"], f32)
    wa = np.asarray(inputs["wa"], f32)
    wb = np.asarray(inputs["wb"], f32)

    perm = np.empty(C2, np.int64)
    sva = np.empty(C2, f32)
    svb = np.empty(C2, f32)
    for h in range(H):
        for br in range(2):
            j0 = h * 64 + br * 32
            perm[j0:j0 + 32] = br * C + h * 32 + np.arange(32)
            sva[j0:j0 + 32] = wa[br] * SCALE
            svb[j0:j0 + 32] = wb[br] * SCALE

    wq_p = np.asarray(inputs["Wq_lf"], f32)[:, perm]
    bq_p = np.asarray(inputs["bq_lf"], f32)[perm]
    wkag_p = np.asarray(inputs["Wk_ag"], f32)[:, perm] * sva[None, :]
    bkag_p = np.asarray(inputs["bk_ag"], f32)[perm] * sva
    wqag_p = np.asarray(inputs["Wq_ag"], f32)[:, perm]
    bqag_p = np.asarray(inputs["bq_ag"], f32)[perm]
    wkhf_p = np.asarray(inputs["Wk_hf"], f32)[:, perm] * svb[None, :]

    wv_in = np.asarray(inputs["Wv_hf"], f32)
    bv_in = np.asarray(inputs["bv_hf"], f32)
    wv_aug = np.zeros((C, H * 33), f32)
    bv_aug = np.zeros(H * 33, f32)
    for h in range(H):
        wv_aug[:, h * 33:h * 33 + 32] = wv_in[:, h * 32:h * 32 + 32]
        bv_aug[h * 33:h * 33 + 32] = bv_in[h * 32:h * 32 + 32]
        bv_aug[h * 33 + 32] = 1.0

    bab = np.array([np.asarray(inputs["ba"], f32)[0],
                    np.asarray(inputs["bb"], f32)[0]], f32)

    shared = {
        "wqT": np.ascontiguousarray(wq_p.T).astype(NPBF16),
        "wkhfT": np.ascontiguousarray(wkhf_p.T).astype(NPBF16),
        "wkag": wkag_p.astype(NPBF16),
        "wqag": wqag_p.astype(NPBF16),
        "wv": wv_aug.astype(NPBF16),
        "wproj": np.asarray(inputs["Wproj"], f32).astype(NPBF16),
        "bq": bq_p, "bkag": bkag_p, "bqag": bqag_p,
        "bv": bv_aug, "bproj": np.ascontiguousarray(np.asarray(inputs["bproj"], f32)),
        "bab": bab,
    }
    xT = np.ascontiguousarray(x.transpose(0, 2, 1)).astype(NPBF16)
    attnT = np.ascontiguousarray(attn.transpose(0, 2, 1)).astype(NPBF16)
    agT = np.ascontiguousarray(agent.transpose(0, 2, 1)).astype(NPBF16)
    in_maps = []
    for b in range(B):
        m = dict(shared)
        m["xT"] = xT[b]
        m["attnT"] = attnT[b]
        m["agT"] = agT[b]
        in_maps.append(m)
    return in_maps


def kernel(**inputs):
    zb = all(not np.any(np.asarray(inputs[k]))
             for k in ("bq_lf", "bk_ag", "bq_ag", "bk_hf", "bv_hf", "bproj",
                       "ba", "bb"))
    key = ("nc", zb)
    if key not in _CACHE:
        _CACHE[key] = _build_bass(zero_bias=zb)
    nc = _CACHE[key]
    in_maps = _prep_host(inputs)
    res = run_bass_kernel_spmd(nc, in_maps, core_ids=list(range(B)))
    return np.stack([res.results[b]["out"] for b in range(B)], axis=0)



# revision 15
# speedup vs baseline: 1.3410x; 1.3410x over previous
"""Trainium2 Bass kernel for the dual-branch agent-attention module.

Sharding: data-parallel over B=8 (one batch element per NeuronCore).

All agent-side math is folded HOST-side: the effective score weights
  WeffA = Wq_p @ blockdiag(K_agents)   (branch A, per batch)
  WeffB = Wkhf_p @ blockdiag(QA_agents) (branch B, per batch)
are computed in numpy (they only involve the 64 agents), quantized to
fp8-e4m3 with a per-tensor dynamic scale, and shipped to the device.
The device then runs a pure streaming pipeline per batch element:

  phase B:   v = attnT^T@Wv (bf16, with ones col for the softmax denom),
             t-scores from attnT8 via WeffB8 (fp8 DoubleRow + fp8 tail)
             -> exp (descale via ACT scale) -> xs accumulated across the
             whole phase in one persistent PSUM accumulation group.
  phase AC:  s-scores from xT8 via WeffA8 (fp8 DR) -> exp -> PA,
             x_out with ones-column denom, normalize, PE-transpose,
             proj (bf16), output stored bf16 (upcast host-side).

Head-major layout trick: projection outputs are permuted host-side from
(branch, head, d) to (head, branch, d) so each head pair occupies one
128-partition tile; branch score scales (wa/wb * D^-0.5) are folded into
the K-side weights so both branches' score maps come from one contraction.

Bias handling: k-side biases constant along a softmax axis cancel and are
dropped (bk_hf); branch-A q-side bias survives as the per-agent term
cA = bq_p @ blockdiag(K), applied as the exp's per-partition bias.
"""

import os
import sys
import numpy as np

for _p in ("/opt/trn_rl_repo", os.path.expanduser("~/.axon_site/_ro/trn_rl_repo")):
    if os.path.isdir(_p) and _p not in sys.path:
        sys.path.insert(0, _p)

import ml_dtypes

import concourse.bass as bass
import concourse.bacc as bacc
import concourse.tile as tile
from concourse import mybir
from concourse.bass_utils import run_bass_kernel_spmd
from concourse.masks import make_identity

BF16 = mybir.dt.bfloat16
FP8 = mybir.dt.float8e4
F32 = mybir.dt.float32
NPBF16 = ml_dtypes.bfloat16
NPFP8 = ml_dtypes.float8_e4m3

B, N, NA, H, D = 8, 4096, 64, 12, 32
C = H * D            # 384
C2 = 2 * C           # 768
CH = 512             # seq chunk
NCH = N // CH        # 8
TPC = CH // 128      # 4 seq tiles per chunk
SCALE = D ** -0.5
DR = mybir.MatmulPerfMode.DoubleRow

_CACHE = {}


def _build_bass(finalize=True, zero_bias=False):
    nc = bacc.Bacc()

    # ---- DRAM I/O ----
    xT8 = nc.dram_tensor("xT8", [C, N], FP8, kind="ExternalInput")
    attnT = nc.dram_tensor("attnT", [C, N], BF16, kind="ExternalInput")
    attnT8 = nc.dram_tensor("attnT8", [C, N], FP8, kind="ExternalInput")
    weffa8 = nc.dram_tensor("weffa8", [C, C2], FP8, kind="ExternalInput")
    weffb8 = nc.dram_tensor("weffb8", [C, C2], FP8, kind="ExternalInput")
    wv = nc.dram_tensor("wv", [C, H * 33], BF16, kind="ExternalInput")
    wproj = nc.dram_tensor("wproj", [C, C], BF16, kind="ExternalInput")
    sc = nc.dram_tensor("sc", [4], F32, kind="ExternalInput")
    cba = nc.dram_tensor("cba", [C2], F32, kind="ExternalInput")
    bv = nc.dram_tensor("bv", [H * 33], F32, kind="ExternalInput")
    bproj = nc.dram_tensor("bproj", [C], F32, kind="ExternalInput")
    out = nc.dram_tensor("out", [N, C], BF16, kind="ExternalOutput")

    Exp = mybir.ActivationFunctionType.Exp

    with tile.TileContext(nc) as tc:
        with (
            tc.tile_pool(name="const", bufs=1) as const,
            tc.tile_pool(name="inp", bufs=3) as p_in,
            tc.tile_pool(name="in8", bufs=3) as p_in8,
            tc.tile_pool(name="inx", bufs=2) as p_inx,
            tc.tile_pool(name="vv", bufs=2) as p_v,
            tc.tile_pool(name="pt", bufs=5) as p_pt,
            tc.tile_pool(name="pa", bufs=2) as p_pa,
            tc.tile_pool(name="xon", bufs=2) as p_xon,
            tc.tile_pool(name="xot", bufs=4) as p_xot,
            tc.tile_pool(name="osb", bufs=3) as p_out,
            tc.tile_pool(name="sm", bufs=4) as p_sm,
        ):
            # ---- constants (small; PE work does not wait on most of them) ----
            w_v = const.tile([128, 3, H * 33], BF16)
            nc.sync.dma_start(out=w_v, in_=wv.rearrange("(k p) m -> p k m", p=128))
            w_b8 = const.tile([128, 3, C2], FP8)
            nc.sync.dma_start(out=w_b8, in_=weffb8.rearrange("(k p) m -> p k m", p=128))
            w_a8 = const.tile([128, 3, C2], FP8)
            nc.sync.dma_start(out=w_a8, in_=weffa8.rearrange("(k p) m -> p k m", p=128))
            w_pr = const.tile([128, 3, C], BF16)
            nc.sync.dma_start(out=w_pr, in_=wproj.rearrange("(k p) m -> p k m", p=128))

            def bcast_scalar(name, offset):
                t = const.tile([128, 1], F32, name=name, tag=name)
                nc.sync.dma_start(out=t, in_=bass.AP(tensor=sc[:].tensor,
                                                       offset=offset,
                                                       ap=[[0, 128], [1, 1]]))
                return t

            sa_t = bcast_scalar("sa", 0)
            sb_t = bcast_scalar("sb", 1)
            ones_row = const.tile([1, 128], BF16)
            nc.vector.memset(ones_row, 1.0)
            ones12 = const.tile([1, 12], BF16)
            nc.vector.memset(ones12, 1.0)
            ident = const.tile([128, 128], BF16)
            make_identity(nc, ident)
            cba_t = None
            if not zero_bias:
                cba_t = const.tile([128, 6], F32)
                nc.sync.dma_start(out=cba_t,
                                    in_=cba.rearrange("(j p) -> p j", p=128))
                bb_t = bcast_scalar("bb", 2)
                bv_row = const.tile([1, H * 33], BF16)
                nc.sync.dma_start(out=bv_row, in_=bv[:].unsqueeze(0))
                bpr_row = const.tile([1, C], BF16)
                nc.sync.dma_start(out=bpr_row, in_=bproj[:].unsqueeze(0))
                # pre-touch so wide ACT ops don't carry the DMA wait
                touch = const.tile([128, 4], F32)
                nc.vector.tensor_copy(touch[:, 0:1], cba_t[:, 0:1])
                nc.vector.tensor_copy(touch[:, 1:2], bb_t[:, 0:1])
            touch2 = const.tile([128, 2], F32)
            nc.vector.tensor_copy(touch2[:, 0:1], sa_t[:, 0:1])
            nc.vector.tensor_copy(touch2[:, 1:2], sb_t[:, 0:1])

            # ---- phase B: values + branch-B attention ----
            # xs accumulates over the WHOLE phase in one PSUM bank.
            pb = tc.tile_pool(name="psBv", bufs=2, space="PSUM")
            p4 = tc.tile_pool(name="psS4", bufs=2, space="PSUM")
            p2 = tc.tile_pool(name="psS2", bufs=2, space="PSUM")
            px = tc.tile_pool(name="psX", bufs=2, space="PSUM")
            psB, psC4, psC2, psX = (pb.__enter__(), p4.__enter__(),
                                    p2.__enter__(), px.__enter__())
            xs_sb = const.tile([128, 6 * 66], F32)
            nc.vector.memset(xs_sb, 0.0)
            for c in range(NCH):
                xs_acc = psX.tile([128, 512], F32, tag="xs")  # own bank
                at_t = p_in.tile([128, 3, CH], BF16, tag="inp")
                nc.sync.dma_start(
                    out=at_t,
                    in_=attnT.rearrange("(k p) s -> p k s", p=128)[:, :, c * CH:(c + 1) * CH])
                at8 = p_in8.tile([128, 3, CH], FP8, tag="inp8")
                nc.sync.dma_start(
                    out=at8,
                    in_=attnT8.rearrange("(k p) s -> p k s", p=128)[:, :, c * CH:(c + 1) * CH])
                v_t = p_v.tile([128, TPC, H * 33], BF16)
                for t in range(TPC):
                    psf = psB.tile([128, 512], F32, tag="mid")
                    ps = psf[:, 0:H * 33]
                    for k in range(3):
                        nc.tensor.matmul(ps, lhsT=at_t[:, k, t * 128:(t + 1) * 128],
                                         rhs=w_v[:, k, :], start=(k == 0),
                                         stop=(zero_bias and k == 2))
                    if not zero_bias:
                        nc.tensor.matmul(ps, lhsT=ones_row[:, :], rhs=bv_row[:, :],
                                         start=False, stop=True)
                    nc.vector.tensor_copy(v_t[:, t, :], ps)
                    if zero_bias:
                        v33 = v_t[:, t, :].rearrange("p (h c) -> p h c", c=33)
                        nc.gpsimd.memset(v33[:, :, 32:33], 1.0)
                for t in range(TPC):
                    ps4 = psC4.tile([128, 512], F32, tag="s4")
                    ps2f = psC2.tile([128, 512], F32, tag="s2")
                    ps2 = ps2f[:, 0:256]
                    nc.tensor.matmul(ps4, lhsT=at8[:, 0:2, t * 128:(t + 1) * 128],
                                     rhs=w_b8[:, 0:2, 0:512], perf_mode=DR,
                                     start=True, stop=False)
                    nc.tensor.matmul(ps4, lhsT=at8[:, 2, t * 128:(t + 1) * 128],
                                     rhs=w_b8[:, 2, 0:512], start=False, stop=True)
                    nc.tensor.matmul(ps2, lhsT=at8[:, 0:2, t * 128:(t + 1) * 128],
                                     rhs=w_b8[:, 0:2, 512:768], perf_mode=DR,
                                     start=True, stop=False)
                    nc.tensor.matmul(ps2, lhsT=at8[:, 2, t * 128:(t + 1) * 128],
                                     rhs=w_b8[:, 2, 512:768], start=False, stop=True)
                    pt4 = p_pt.tile([128, 512], BF16, tag="pt4")
                    pt2 = p_pt.tile([128, 256], BF16, tag="pt2")
                    bbias = 0.0 if zero_bias else bb_t[:, 0:1]
                    nc.scalar.activation(pt4, ps4, Exp, bias=bbias,
                                         scale=sb_t[:, 0:1])
                    nc.scalar.activation(pt2, ps2, Exp, bias=bbias,
                                         scale=sb_t[:, 0:1])
                    for j in range(6):
                        lhsT = pt4[:, j * 128:(j + 1) * 128] if j < 4 else \
                            pt2[:, (j - 4) * 128:(j - 3) * 128]
                        nc.tensor.matmul(xs_acc[:, j * 66:(j + 1) * 66], lhsT=lhsT,
                                         rhs=v_t[:, t, j * 66:(j + 1) * 66],
                                         start=(t == 0 and j == 0),
                                         stop=(t == TPC - 1 and j == 5),
                                         skip_group_check=True)
                nc.vector.tensor_add(xs_sb, xs_sb, xs_acc[:, 0:396])

            # ---- xs normalize -> block-diag [xs | 1] tiles ----
            xs_bd = const.tile([128, 6 * 66], BF16)
            xs3 = xs_sb[:].rearrange("p (j c) -> p j c", c=66)
            bd3 = xs_bd[:].rearrange("p (j c) -> p j c", c=66)
            nc.vector.memset(xs_bd, 0.0)
            nc.vector.memset(bd3[0:64, :, 32:33], 1.0)
            nc.vector.memset(bd3[64:128, :, 65:66], 1.0)
            rec6 = p_sm.tile([128, 6], F32, tag="rec")
            nc.vector.reciprocal(rec6[0:64, :], xs3[0:64, :, 32])
            nc.vector.reciprocal(rec6[64:128, :], xs3[64:128, :, 65])
            nc.vector.tensor_mul(bd3[0:64, :, 0:32], xs3[0:64, :, 0:32],
                                 rec6[0:64, :].unsqueeze(2).to_broadcast([64, 6, 32]))
            nc.vector.tensor_mul(bd3[64:128, :, 33:65], xs3[64:128, :, 33:65],
                                 rec6[64:128, :].unsqueeze(2).to_broadcast([64, 6, 32]))
            px.__exit__(None, None, None)
            p2.__exit__(None, None, None)
            p4.__exit__(None, None, None)
            pb.__exit__(None, None, None)

            # ---- phase AC: branch-A attention + proj ----
            pa_ = tc.tile_pool(name="psA", bufs=2, space="PSUM")
            pxo = tc.tile_pool(name="psXO", bufs=2, space="PSUM")
            ppr = tc.tile_pool(name="psPR", bufs=2, space="PSUM")
            ptp = tc.tile_pool(name="psTP", bufs=2, space="PSUM")
            psA, psXO, psPR, psTP = (pa_.__enter__(), pxo.__enter__(),
                                     ppr.__enter__(), ptp.__enter__())
            for c in range(NCH):
                xt8 = p_inx.tile([128, 3, CH], FP8, tag="inx")
                nc.sync.dma_start(
                    out=xt8,
                    in_=xT8.rearrange("(k p) s -> p k s", p=128)[:, :, c * CH:(c + 1) * CH])
                pa_t = p_pa.tile([128, 6, CH], BF16)
                for j in range(6):
                    ps = psA.tile([128, CH], F32, tag="big")
                    nc.tensor.matmul(ps, lhsT=w_a8[:, 0:2, j * 128:(j + 1) * 128],
                                     rhs=xt8[:, 0:2, :], perf_mode=DR,
                                     start=True, stop=False)
                    nc.tensor.matmul(ps, lhsT=w_a8[:, 2, j * 128:(j + 1) * 128],
                                     rhs=xt8[:, 2, :], start=False, stop=True)
                    nc.scalar.activation(pa_t[:, j, :], ps, Exp,
                                         bias=(0.0 if zero_bias else cba_t[:, j:j + 1]),
                                         scale=sa_t[:, 0:1])
                for t in range(TPC):
                    xof = psXO.tile([128, 512], F32, tag="mid")
                    xo_ps = xof[:, 0:12 * 33]
                    for j in range(6):
                        nc.tensor.matmul(xo_ps[:, j * 66:(j + 1) * 66],
                                         lhsT=pa_t[:, j, t * 128:(t + 1) * 128],
                                         rhs=xs_bd[:, j * 66:(j + 1) * 66],
                                         start=True, stop=True)
                    xo3 = xo_ps.rearrange("p (k c) -> p k c", c=33)
                    rec = p_sm.tile([128, 12], F32, tag="rec12")
                    nc.vector.reciprocal(rec, xo3[:, :, 32])
                    xon = p_xon.tile([128, C], BF16)
                    nc.vector.tensor_mul(xon[:].rearrange("p (k c) -> p k c", c=32),
                                         xo3[:, :, 0:32],
                                         rec[:].unsqueeze(2).to_broadcast([128, 12, 32]))
                    prf = psPR.tile([128, 512], F32, tag="pr")
                    pr_ps = prf[:, 0:C]
                    for f in range(3):
                        tpf = psTP.tile([128, 1024], BF16, tag="tp")
                        tp = tpf[:, 0:128]
                        nc.tensor.transpose(tp, xon[:, f * 128:(f + 1) * 128], ident)
                        xot = p_xot.tile([128, 128], BF16)
                        nc.vector.tensor_copy(xot, tp)
                        nc.tensor.matmul(pr_ps, lhsT=xot, rhs=w_pr[:, f, :],
                                         start=(f == 0),
                                         stop=(zero_bias and f == 2),
                                         skip_group_check=True)
                    if not zero_bias:
                        nc.tensor.matmul(pr_ps, lhsT=ones_row[:, :], rhs=bpr_row[:, :],
                                         start=False, stop=True, skip_group_check=True)
                    o_sb = p_out.tile([128, C], BF16)
                    nc.scalar.copy(o_sb, pr_ps)
                    r0 = (c * TPC + t) * 128
                    nc.sync.dma_start(out=out[r0:r0 + 128, :], in_=o_sb)
            ptp.__exit__(None, None, None)
            ppr.__exit__(None, None, None)
            pxo.__exit__(None, None, None)
            pa_.__exit__(None, None, None)
    if finalize:
        nc.finalize()
    return nc


def _prep_host(inputs):
    f32 = np.float32
    x = np.asarray(inputs["x"], f32)
    attn = np.asarray(inputs["attn"], f32)
    agent = np.asarray(inputs["agent_input"], f32)
    wa = np.asarray(inputs["wa"], f32)
    wb = np.asarray(inputs["wb"], f32)
    ba = np.asarray(inputs["ba"], f32)
    bb = np.asarray(inputs["bb"], f32)

    perm = np.empty(C2, np.int64)
    sva = np.empty(C2, f32)
    svb = np.empty(C2, f32)
    for h in range(H):
        for br in range(2):
            j0 = h * 64 + br * 32
            perm[j0:j0 + 32] = br * C + h * 32 + np.arange(32)
            sva[j0:j0 + 32] = wa[br] * SCALE
            svb[j0:j0 + 32] = wb[br] * SCALE

    wq_p = np.asarray(inputs["Wq_lf"], f32)[:, perm]
    bq_p = np.asarray(inputs["bq_lf"], f32)[perm]
    wkag_p = np.asarray(inputs["Wk_ag"], f32)[:, perm] * sva[None, :]
    bkag_p = np.asarray(inputs["bk_ag"], f32)[perm] * sva
    wqag_p = np.asarray(inputs["Wq_ag"], f32)[:, perm]
    bqag_p = np.asarray(inputs["bq_ag"], f32)[perm]
    wkhf_p = np.asarray(inputs["Wk_hf"], f32)[:, perm] * svb[None, :]
    # bk_hf is constant along the branch-B softmax axis -> cancels, dropped.

    wv_in = np.asarray(inputs["Wv_hf"], f32)
    bv_in = np.asarray(inputs["bv_hf"], f32)
    wv_aug = np.zeros((C, H * 33), f32)
    bv_aug = np.zeros(H * 33, f32)
    for h in range(H):
        wv_aug[:, h * 33:h * 33 + 32] = wv_in[:, h * 32:h * 32 + 32]
        bv_aug[h * 33:h * 33 + 32] = bv_in[h * 32:h * 32 + 32]
        bv_aug[h * 33 + 32] = 1.0

    shared = {
        "wv": wv_aug.astype(NPBF16),
        "wproj": np.asarray(inputs["Wproj"], f32).astype(NPBF16),
        "bv": bv_aug,
        "bproj": np.ascontiguousarray(np.asarray(inputs["bproj"], f32)),
    }

    xT = np.ascontiguousarray(x.transpose(0, 2, 1))
    attnT = np.ascontiguousarray(attn.transpose(0, 2, 1))
    xT8 = np.clip(xT, -240, 240).astype(NPFP8)
    attnT_bf = attnT.astype(NPBF16)
    attnT8 = np.clip(attnT, -240, 240).astype(NPFP8)

    in_maps = []
    for b in range(B):
        ag = agent[b]                              # [64, 384]
        K_b = ag @ wkag_p + bkag_p                 # [64, 768] (wa*SCALE folded)
        QA_b = ag @ wqag_p + bqag_p                # [64, 768]
        weffA = np.empty((C, C2), f32)
        weffB = np.empty((C, C2), f32)
        cA = np.empty(C2, f32)
        for h in range(H):
            s = slice(h * 64, (h + 1) * 64)
            weffA[:, s] = wq_p[:, s] @ K_b[:, s].T
            weffB[:, s] = wkhf_p[:, s] @ QA_b[:, s].T
            cA[s] = bq_p[s] @ K_b[:, s].T
        cA += ba[0]
        swa = 224.0 / max(float(np.abs(weffA).max()), 1e-30)
        swb = 224.0 / max(float(np.abs(weffB).max()), 1e-30)
        m = dict(shared)
        m["weffa8"] = np.clip(weffA * swa, -240, 240).astype(NPFP8)
        m["weffb8"] = np.clip(weffB * swb, -240, 240).astype(NPFP8)
        m["sc"] = np.array([1.0 / swa, 1.0 / swb, bb[0], 0.0], f32)
        m["cba"] = cA
        m["xT8"] = xT8[b]
        m["attnT"] = attnT_bf[b]
        m["attnT8"] = attnT8[b]
        in_maps.append(m)
    return in_maps


def kernel(**inputs):
    zb = all(not np.any(np.asarray(inputs[k]))
             for k in ("bq_lf", "bk_ag", "bq_ag", "bk_hf", "bv_hf", "bproj",
                       "ba", "bb"))
    key = ("nc", zb)
    if key not in _CACHE:
        _CACHE[key] = _build_bass(zero_bias=zb)
    nc = _CACHE[key]
    in_maps = _prep_host(inputs)
    res = run_bass_kernel_spmd(nc, in_maps, core_ids=list(range(B)))
    return np.stack([res.results[b]["out"].astype(np.float32) for b in range(B)],
                    axis=0)


# revision 21
# speedup vs baseline: 1.3781x; 1.0276x over previous
"""Trainium2 Bass kernel for the dual-branch agent-attention module.

Sharding: data-parallel over B=8 (one batch element per NeuronCore).

All agent-side math is folded HOST-side: the effective score weights
  WeffA = Wq_p @ blockdiag(K_agents)   (branch A, per batch)
  WeffB = Wkhf_p @ blockdiag(QA_agents) (branch B, per batch)
are computed in numpy (they only involve the 64 agents), quantized to
fp8-e4m3 with a per-tensor dynamic scale, and shipped to the device.
The device then runs a pure streaming pipeline per batch element:

  phase B:   v = attnT^T@Wv (bf16, with ones col for the softmax denom),
             t-scores from attnT8 via WeffB8 (fp8 DoubleRow + fp8 tail)
             -> exp (descale via ACT scale) -> xs accumulated across the
             whole phase in one persistent PSUM accumulation group.
  phase AC:  s-scores from xT8 via WeffA8 (fp8 DR) -> exp -> PA,
             x_out with ones-column denom, normalize, PE-transpose,
             proj (bf16), output stored bf16 (upcast host-side).

Head-major layout trick: projection outputs are permuted host-side from
(branch, head, d) to (head, branch, d) so each head pair occupies one
128-partition tile; branch score scales (wa/wb * D^-0.5) are folded into
the K-side weights so both branches' score maps come from one contraction.

Bias handling: k-side biases constant along a softmax axis cancel and are
dropped (bk_hf); branch-A q-side bias survives as the per-agent term
cA = bq_p @ blockdiag(K), applied as the exp's per-partition bias.
"""

import os
import sys
import numpy as np

for _p in ("/opt/trn_rl_repo", os.path.expanduser("~/.axon_site/_ro/trn_rl_repo")):
    if os.path.isdir(_p) and _p not in sys.path:
        sys.path.insert(0, _p)

import ml_dtypes

import concourse.bass as bass
import concourse.bacc as bacc
import concourse.tile as tile
from concourse import mybir
from concourse.bass_utils import run_bass_kernel_spmd
from concourse.masks import make_identity

BF16 = mybir.dt.bfloat16
FP8 = mybir.dt.float8e4
F32 = mybir.dt.float32
NPBF16 = ml_dtypes.bfloat16
NPFP8 = ml_dtypes.float8_e4m3

B, N, NA, H, D = 8, 4096, 64, 12, 32
C = H * D            # 384
C2 = 2 * C           # 768
CH = 512             # seq chunk
NCH = N // CH        # 8
TPC = CH // 128      # 4 seq tiles per chunk
SCALE = D ** -0.5
DR = mybir.MatmulPerfMode.DoubleRow

_CACHE = {}


def _build_bass(finalize=True, zero_bias=False):
    nc = bacc.Bacc()

    # ---- DRAM I/O ----
    xT8 = nc.dram_tensor("xT8", [NCH, 128, 3, CH], FP8, kind="ExternalInput")
    attnT = nc.dram_tensor("attnT", [NCH, 128, 3, CH], BF16, kind="ExternalInput")
    attnT8 = nc.dram_tensor("attnT8", [NCH, 128, 3, CH], FP8, kind="ExternalInput")
    weffa8 = nc.dram_tensor("weffa8", [C, C2], FP8, kind="ExternalInput")
    weffb8 = nc.dram_tensor("weffb8", [C, C2], FP8, kind="ExternalInput")
    wv = nc.dram_tensor("wv", [C, H * 33], BF16, kind="ExternalInput")
    wproj = nc.dram_tensor("wproj", [C, C], BF16, kind="ExternalInput")
    sc = nc.dram_tensor("sc", [4], F32, kind="ExternalInput")
    cba = nc.dram_tensor("cba", [C2], F32, kind="ExternalInput")
    bv = nc.dram_tensor("bv", [H * 33], F32, kind="ExternalInput")
    bproj = nc.dram_tensor("bproj", [C], F32, kind="ExternalInput")
    out = nc.dram_tensor("out", [N, C], BF16, kind="ExternalOutput")

    Exp = mybir.ActivationFunctionType.Exp

    with tile.TileContext(nc) as tc:
        with (
            tc.tile_pool(name="const", bufs=1) as const,
            tc.tile_pool(name="inp", bufs=3) as p_in,
            tc.tile_pool(name="in8", bufs=3) as p_in8,
            tc.tile_pool(name="inx", bufs=2) as p_inx,
            tc.tile_pool(name="vv", bufs=2) as p_v,
            tc.tile_pool(name="pt", bufs=5) as p_pt,
            tc.tile_pool(name="pa", bufs=2) as p_pa,
            tc.tile_pool(name="xon", bufs=2) as p_xon,
            tc.tile_pool(name="xot", bufs=4) as p_xot,
            tc.tile_pool(name="osb", bufs=3) as p_out,
            tc.tile_pool(name="sm", bufs=4) as p_sm,
        ):
            # ---- constants (small; PE work does not wait on most of them) ----
            w_v = const.tile([128, 3, H * 33], BF16)
            nc.sync.dma_start(out=w_v, in_=wv.rearrange("(k p) m -> p k m", p=128))
            w_b8 = const.tile([128, 3, C2], FP8)
            nc.sync.dma_start(out=w_b8, in_=weffb8.rearrange("(k p) m -> p k m", p=128))
            w_a8 = const.tile([128, 3, C2], FP8)
            w_pr = const.tile([128, 3, C], BF16)

            def bcast_scalar(name, offset):
                t = const.tile([128, 1], F32, name=name, tag=name)
                nc.sync.dma_start(out=t, in_=bass.AP(tensor=sc[:].tensor,
                                                       offset=offset,
                                                       ap=[[0, 128], [1, 1]]))
                return t

            sa_t = bcast_scalar("sa", 0)
            sb_t = bcast_scalar("sb", 1)
            ones_row = const.tile([1, 128], BF16)
            nc.vector.memset(ones_row, 1.0)
            ones12 = const.tile([1, 12], BF16)
            nc.vector.memset(ones12, 1.0)
            ident = const.tile([128, 128], BF16)
            make_identity(nc, ident)
            cba_t = None
            if not zero_bias:
                cba_t = const.tile([128, 6], F32)
                nc.sync.dma_start(out=cba_t,
                                    in_=cba.rearrange("(j p) -> p j", p=128))
                bb_t = bcast_scalar("bb", 2)
                bv_row = const.tile([1, H * 33], BF16)
                nc.sync.dma_start(out=bv_row, in_=bv[:].unsqueeze(0))
                bpr_row = const.tile([1, C], BF16)
                nc.sync.dma_start(out=bpr_row, in_=bproj[:].unsqueeze(0))
                # pre-touch so wide ACT ops don't carry the DMA wait
                touch = const.tile([128, 4], F32)
                nc.vector.tensor_copy(touch[:, 0:1], cba_t[:, 0:1])
                nc.vector.tensor_copy(touch[:, 1:2], bb_t[:, 0:1])
            touch2 = const.tile([128, 2], F32)
            nc.vector.tensor_copy(touch2[:, 0:1], sa_t[:, 0:1])
            nc.vector.tensor_copy(touch2[:, 1:2], sb_t[:, 0:1])

            # ---- phase B: values + branch-B attention ----
            # xs accumulates over the WHOLE phase in one PSUM bank.
            pb = tc.tile_pool(name="psBv", bufs=2, space="PSUM")
            p4 = tc.tile_pool(name="psS", bufs=2, space="PSUM")
            px = tc.tile_pool(name="psX", bufs=2, space="PSUM")
            psB, psS, psX = (pb.__enter__(), p4.__enter__(), px.__enter__())
            xs_sb = const.tile([128, 6 * 66], F32)
            nc.vector.memset(xs_sb, 0.0)
            for c in range(NCH):
                xs_acc = psX.tile([128, 512], F32, tag="xs")  # own bank
                at_t = p_in.tile([128, 3, CH], BF16, tag="inp")
                nc.sync.dma_start(out=at_t, in_=attnT[c])
                at8 = p_in8.tile([128, 3, CH], FP8, tag="inp8")
                nc.sync.dma_start(out=at8, in_=attnT8[c])
                v_t = p_v.tile([128, TPC, H * 33], BF16)
                for t in range(TPC):
                    psf = psB.tile([128, 512], F32, tag="mid")
                    ps = psf[:, 0:H * 33]
                    for k in range(3):
                        nc.tensor.matmul(ps, lhsT=at_t[:, k, t * 128:(t + 1) * 128],
                                         rhs=w_v[:, k, :], start=(k == 0),
                                         stop=(zero_bias and k == 2))
                    if not zero_bias:
                        nc.tensor.matmul(ps, lhsT=ones_row[:, :], rhs=bv_row[:, :],
                                         start=False, stop=True)
                    nc.vector.tensor_copy(v_t[:, t, :], ps)
                    if zero_bias:
                        v33 = v_t[:, t, :].rearrange("p (h c) -> p h c", c=33)
                        nc.gpsimd.memset(v33[:, :, 32:33], 1.0)
                pts = []

                def emit_scores(t):
                    psw = psS.tile([128, 1024], F32, tag="s", name=f"psw{t}")
                    ps4 = psw[:, 0:512]
                    ps2 = psw[:, 512:768]
                    lhs01 = at8[:, 0:2, t * 128:(t + 1) * 128]
                    lhs2 = at8[:, 2, t * 128:(t + 1) * 128]
                    nc.tensor.matmul(ps4, lhsT=lhs01, rhs=w_b8[:, 0:2, 0:512],
                                     perf_mode=DR, start=True, stop=False,
                                     skip_group_check=True)
                    nc.tensor.matmul(ps2, lhsT=lhs01, rhs=w_b8[:, 0:2, 512:768],
                                     perf_mode=DR, start=True, stop=False,
                                     skip_group_check=True)
                    nc.tensor.matmul(ps4, lhsT=lhs2, rhs=w_b8[:, 2, 0:512],
                                     start=False, stop=True, skip_group_check=True)
                    nc.tensor.matmul(ps2, lhsT=lhs2, rhs=w_b8[:, 2, 512:768],
                                     start=False, stop=True, skip_group_check=True)
                    pt = p_pt.tile([128, 768], BF16, tag="pt", name=f"pt{t}")
                    bbias = 0.0 if zero_bias else bb_t[:, 0:1]
                    nc.scalar.activation(pt, psw[:, 0:768], Exp, bias=bbias,
                                         scale=sb_t[:, 0:1])
                    pts.append(pt)

                def emit_xs(t):
                    pt = pts[t]
                    for j in range(6):
                        nc.tensor.matmul(xs_acc[:, j * 66:(j + 1) * 66],
                                         lhsT=pt[:, j * 128:(j + 1) * 128],
                                         rhs=v_t[:, t, j * 66:(j + 1) * 66],
                                         start=(t == 0 and j == 0),
                                         stop=(t == TPC - 1 and j == 5),
                                         skip_group_check=True)

                emit_scores(0)
                emit_scores(1)
                emit_xs(0)
                emit_scores(2)
                emit_xs(1)
                emit_scores(3)
                emit_xs(2)
                emit_xs(3)
                nc.vector.tensor_add(xs_sb, xs_sb, xs_acc[:, 0:396])

            # ---- xs normalize -> block-diag [xs | 1] tiles ----
            xs_bd = const.tile([128, 6 * 66], BF16)
            xs3 = xs_sb[:].rearrange("p (j c) -> p j c", c=66)
            bd3 = xs_bd[:].rearrange("p (j c) -> p j c", c=66)
            nc.vector.memset(xs_bd, 0.0)
            nc.vector.memset(bd3[0:64, :, 32:33], 1.0)
            nc.vector.memset(bd3[64:128, :, 65:66], 1.0)
            rec6 = p_sm.tile([128, 6], F32, tag="rec")
            nc.vector.reciprocal(rec6[0:64, :], xs3[0:64, :, 32])
            nc.vector.reciprocal(rec6[64:128, :], xs3[64:128, :, 65])
            nc.vector.tensor_mul(bd3[0:64, :, 0:32], xs3[0:64, :, 0:32],
                                 rec6[0:64, :].unsqueeze(2).to_broadcast([64, 6, 32]))
            nc.vector.tensor_mul(bd3[64:128, :, 33:65], xs3[64:128, :, 33:65],
                                 rec6[64:128, :].unsqueeze(2).to_broadcast([64, 6, 32]))
            px.__exit__(None, None, None)
            p4.__exit__(None, None, None)
            pb.__exit__(None, None, None)

            # ---- phase AC: branch-A attention + proj ----
            nc.sync.dma_start(out=w_a8, in_=weffa8.rearrange("(k p) m -> p k m", p=128))
            nc.sync.dma_start(out=w_pr, in_=wproj.rearrange("(k p) m -> p k m", p=128))
            pa_ = tc.tile_pool(name="psA", bufs=2, space="PSUM")
            pxo = tc.tile_pool(name="psXO", bufs=2, space="PSUM")
            ppr = tc.tile_pool(name="psPR", bufs=2, space="PSUM")
            ptp = tc.tile_pool(name="psTP", bufs=2, space="PSUM")
            psA, psXO, psPR, psTP = (pa_.__enter__(), pxo.__enter__(),
                                     ppr.__enter__(), ptp.__enter__())
            for c in range(NCH):
                xt8 = p_inx.tile([128, 3, CH], FP8, tag="inx")
                nc.sync.dma_start(out=xt8, in_=xT8[c])
                pa_t = p_pa.tile([128, 6, CH], BF16)
                for j in range(6):
                    ps = psA.tile([128, CH], F32, tag="big")
                    nc.tensor.matmul(ps, lhsT=w_a8[:, 0:2, j * 128:(j + 1) * 128],
                                     rhs=xt8[:, 0:2, :], perf_mode=DR,
                                     start=True, stop=False)
                    nc.tensor.matmul(ps, lhsT=w_a8[:, 2, j * 128:(j + 1) * 128],
                                     rhs=xt8[:, 2, :], start=False, stop=True)
                    nc.scalar.activation(pa_t[:, j, :], ps, Exp,
                                         bias=(0.0 if zero_bias else cba_t[:, j:j + 1]),
                                         scale=sa_t[:, 0:1])
                for t in range(TPC):
                    xof = psXO.tile([128, 512], F32, tag="mid")
                    xo_ps = xof[:, 0:12 * 33]
                    for j in range(6):
                        nc.tensor.matmul(xo_ps[:, j * 66:(j + 1) * 66],
                                         lhsT=pa_t[:, j, t * 128:(t + 1) * 128],
                                         rhs=xs_bd[:, j * 66:(j + 1) * 66],
                                         start=True, stop=True)
                    xo3 = xo_ps.rearrange("p (k c) -> p k c", c=33)
                    rec = p_sm.tile([128, 12], F32, tag="rec12")
                    nc.vector.reciprocal(rec, xo3[:, :, 32])
                    xon = p_xon.tile([128, C], BF16)
                    nc.vector.tensor_mul(xon[:].rearrange("p (k c) -> p k c", c=32),
                                         xo3[:, :, 0:32],
                                         rec[:].unsqueeze(2).to_broadcast([128, 12, 32]))
                    prf = psPR.tile([128, 512], F32, tag="pr")
                    pr_ps = prf[:, 0:C]
                    tps = []
                    xots = []

                    def emit_tp(f):
                        tpf = psTP.tile([128, 1024], BF16, tag="tp", name=f"tp{f}")
                        tp = tpf[:, 0:128]
                        nc.tensor.transpose(tp, xon[:, f * 128:(f + 1) * 128], ident)
                        tps.append(tp)

                    def emit_copy(f):
                        xot = p_xot.tile([128, 128], BF16, name=f"xot{f}")
                        nc.vector.tensor_copy(xot, tps[f])
                        xots.append(xot)

                    def emit_mm(f):
                        nc.tensor.matmul(pr_ps, lhsT=xots[f], rhs=w_pr[:, f, :],
                                         start=(f == 0),
                                         stop=(zero_bias and f == 2),
                                         skip_group_check=True)

                    emit_tp(0)
                    emit_tp(1)
                    emit_copy(0)
                    emit_mm(0)
                    emit_tp(2)
                    emit_copy(1)
                    emit_mm(1)
                    emit_copy(2)
                    emit_mm(2)
                    if not zero_bias:
                        nc.tensor.matmul(pr_ps, lhsT=ones_row[:, :], rhs=bpr_row[:, :],
                                         start=False, stop=True, skip_group_check=True)
                    o_sb = p_out.tile([128, C], BF16)
                    nc.scalar.copy(o_sb, pr_ps)
                    r0 = (c * TPC + t) * 128
                    nc.sync.dma_start(out=out[r0:r0 + 128, :], in_=o_sb)
            ptp.__exit__(None, None, None)
            ppr.__exit__(None, None, None)
            pxo.__exit__(None, None, None)
            pa_.__exit__(None, None, None)
    if finalize:
        nc.finalize()
    return nc


def _prep_host(inputs):
    f32 = np.float32
    x = np.asarray(inputs["x"], f32)
    attn = np.asarray(inputs["attn"], f32)
    agent = np.asarray(inputs["agent_input"], f32)
    wa = np.asarray(inputs["wa"], f32)
    wb = np.asarray(inputs["wb"], f32)
    ba = np.asarray(inputs["ba"], f32)
    bb = np.asarray(inputs["bb"], f32)

    perm = np.empty(C2, np.int64)
    sva = np.empty(C2, f32)
    svb = np.empty(C2, f32)
    for h in range(H):
        for br in range(2):
            j0 = h * 64 + br * 32
            perm[j0:j0 + 32] = br * C + h * 32 + np.arange(32)
            sva[j0:j0 + 32] = wa[br] * SCALE
            svb[j0:j0 + 32] = wb[br] * SCALE

    wq_p = np.asarray(inputs["Wq_lf"], f32)[:, perm]
    bq_p = np.asarray(inputs["bq_lf"], f32)[perm]
    wkag_p = np.asarray(inputs["Wk_ag"], f32)[:, perm] * sva[None, :]
    bkag_p = np.asarray(inputs["bk_ag"], f32)[perm] * sva
    wqag_p = np.asarray(inputs["Wq_ag"], f32)[:, perm]
    bqag_p = np.asarray(inputs["bq_ag"], f32)[perm]
    wkhf_p = np.asarray(inputs["Wk_hf"], f32)[:, perm] * svb[None, :]
    # bk_hf is constant along the branch-B softmax axis -> cancels, dropped.

    wv_in = np.asarray(inputs["Wv_hf"], f32)
    bv_in = np.asarray(inputs["bv_hf"], f32)
    wv_aug = np.zeros((C, H * 33), f32)
    bv_aug = np.zeros(H * 33, f32)
    for h in range(H):
        wv_aug[:, h * 33:h * 33 + 32] = wv_in[:, h * 32:h * 32 + 32]
        bv_aug[h * 33:h * 33 + 32] = bv_in[h * 32:h * 32 + 32]
        bv_aug[h * 33 + 32] = 1.0

    shared = {
        "wv": wv_aug.astype(NPBF16),
        "wproj": np.asarray(inputs["Wproj"], f32).astype(NPBF16),
        "bv": bv_aug,
        "bproj": np.ascontiguousarray(np.asarray(inputs["bproj"], f32)),
    }

    def chunk_major(arrT):
        # [C, N] -> [NCH, 128, 3, CH]; element (c,p,k,s) = arrT[k*128+p, c*CH+s]
        return np.ascontiguousarray(
            arrT.reshape(3, 128, NCH, CH).transpose(2, 1, 0, 3))

    xT = x.transpose(0, 2, 1)
    attnT = attn.transpose(0, 2, 1)
    xT8 = np.stack([chunk_major(np.clip(xT[b], -240, 240)).astype(NPFP8)
                    for b in range(B)])
    attnT_bf = np.stack([chunk_major(attnT[b]).astype(NPBF16) for b in range(B)])
    attnT8 = np.stack([chunk_major(np.clip(attnT[b], -240, 240)).astype(NPFP8)
                       for b in range(B)])

    in_maps = []
    for b in range(B):
        ag = agent[b]                              # [64, 384]
        K_b = ag @ wkag_p + bkag_p                 # [64, 768] (wa*SCALE folded)
        QA_b = ag @ wqag_p + bqag_p                # [64, 768]
        weffA = np.empty((C, C2), f32)
        weffB = np.empty((C, C2), f32)
        cA = np.empty(C2, f32)
        for h in range(H):
            s = slice(h * 64, (h + 1) * 64)
            weffA[:, s] = wq_p[:, s] @ K_b[:, s].T
            weffB[:, s] = wkhf_p[:, s] @ QA_b[:, s].T
            cA[s] = bq_p[s] @ K_b[:, s].T
        cA += ba[0]
        swa = 224.0 / max(float(np.abs(weffA).max()), 1e-30)
        swb = 224.0 / max(float(np.abs(weffB).max()), 1e-30)
        m = dict(shared)
        m["weffa8"] = np.clip(weffA * swa, -240, 240).astype(NPFP8)
        m["weffb8"] = np.clip(weffB * swb, -240, 240).astype(NPFP8)
        m["sc"] = np.array([1.0 / swa, 1.0 / swb, bb[0], 0.0], f32)
        m["cba"] = cA
        m["xT8"] = xT8[b]
        m["attnT"] = attnT_bf[b]
        m["attnT8"] = attnT8[b]
        in_maps.append(m)
    return in_maps


def kernel(**inputs):
    zb = all(not np.any(np.asarray(inputs[k]))
             for k in ("bq_lf", "bk_ag", "bq_ag", "bk_hf", "bv_hf", "bproj",
                       "ba", "bb"))
    key = ("nc", zb)
    if key not in _CACHE:
        _CACHE[key] = _build_bass(zero_bias=zb)
    nc = _CACHE[key]
    in_maps = _prep_host(inputs)
    res = run_bass_kernel_spmd(nc, in_maps, core_ids=list(range(B)))
    return np.stack([res.results[b]["out"].astype(np.float32) for b in range(B)],
                    axis=0)
